# revision 38
# baseline (speedup 1.0000x reference)
"""DOAModel (VGG19 taps -> pyramid -> 1600x1600 correlation -> dual softmax ->
top-k -> decoder convs -> bilinear upsample) on 8 TRN2 NeuronCores.

Sharding: 2 cores per sample (batch=4). Each core computes the top half of the
output (rows [0,160) of 320) for its sample; odd cores receive a vertically
flipped input (and vertically flipped conv kernels), so every core runs the
identical SPMD program "compute the top half". The host flips odd-core outputs
back. The correlation stage needs the full 40x40 feature map, so the two cores
of a pair AllGather their normalized feature halves (and later their softmax
column sums) through HBM; everything else is recomputed locally with a halo.

Self-contained: hardcodes all shapes; only imports concourse (the Bass stack)
and numpy.
"""

import math

import numpy as np

import concourse.bass as bass
from concourse import bacc
import concourse.mybir as mybir
import concourse.tile as tile
from concourse.bass_utils import run_bass_kernel_spmd
from concourse.masks import make_identity

F32 = mybir.dt.float32
AF = mybir.ActivationFunctionType
ALU = mybir.AluOpType

P = 128
HW = 40
TOPK = 20
ALPHA = 5.0
BN_EPS = 1e-5

DEBUG_OUTPUTS = False

QROWS = 23            # q rows computed per core (20 own + 3 tail halo)
NQ = QROWS * HW       # 920
NP_ = 2 * 800         # full p axis (rank-ordered)

# Standard conv layer table (after conv0 which is special-cased via im2col).
# name, Cin, Cout, Rout, W, pool, relu, src, dst
# DRAM activation layout: (Groups, Cp, OH+1, W+2) with zero row 0 / cols 0,W+1.
LAYERS = [
    ("c1",  64,   64, 260, 320, True,  True, "a0",   "a1"),
    ("c2",  64,  128, 129, 160, False, True, "a1",   "a2"),
    ("c3", 128,  128, 128, 160, True,  True, "a2",   "a3"),
    ("c4", 128,  256,  63,  80, False, True, "a3",   "a4"),
    ("c5", 256,  256,  62,  80, False, True, "a4",   "a5"),
    ("c6", 256,  256,  61,  80, False, True, "a5",   "a6"),
    ("c7", 256,  256,  60,  80, True,  True, "a6",   "a7"),
    ("c8", 256,  512,  29,  40, False, True, "a7",   "a8"),
    ("c9", 512,  512,  28,  40, False, True, "a8",   "a9"),
    ("c10", 512, 512,  27,  40, False, True, "a9",   "a10"),
    ("c11", 512, 512,  26,  40, True,  True, "a10",  "a11"),
    ("ic", 896,  256,  24,  40, False, True, "feat", "q"),
    ("pyr", 768, 256,  23,  40, False, True, "spp",  "qs"),
]

# DRAM activation tensors: name -> (Groups, Cp, OH, W)  (stored OH+1 x W+2)
ACTS = {
    "a0":  (1, 64, 261, 320),
    "a1":  (1, 64, 130, 160),
    "a2":  (1, 128, 129, 160),
    "a3":  (1, 128, 64, 80),
    "a4":  (2, 128, 63, 80),
    "a5":  (2, 128, 62, 80),
    "a6":  (2, 128, 61, 80),
    "a7":  (2, 128, 30, 40),
    "a8":  (4, 128, 29, 40),
    "a9":  (4, 128, 28, 40),
    "a10": (4, 128, 27, 40),
    "a11": (4, 128, 13, 20),
    "feat": (7, 128, 25, 40),
    "q":   (2, 128, 24, 40),
    "spp": (6, 128, 24, 40),
    "qs":  (2, 128, 23, 40),
}


def _ceil_div(a, b):
    return (a + b - 1) // b


# ---------------------------------------------------------------------------
# kernel builder
# ---------------------------------------------------------------------------

def build_nc(n_cores=8):
    nc = bacc.Bacc(None, num_devices=n_cores, debug=False)
    groups = [[i, i + 1] for i in range(0, n_cores, 2)]

    # ---- DRAM I/O declarations -------------------------------------------
    x0col = nc.dram_tensor("x0col", (27, 261, 320), F32, kind="ExternalInput")
    w_dr = {}
    b_dr = {}

    def decl_w(name, T, Kg, Kp, Cout):
        w_dr[name] = nc.dram_tensor(f"w_{name}", (T, Kg, Kp, Cout), F32,
                                    kind="ExternalInput")
        b_dr[name] = nc.dram_tensor(f"b_{name}", (Cout, 1), F32,
                                    kind="ExternalInput")

    decl_w("c0", 1, 1, 27, 64)
    for (name, Cin, Cout, _, _, _, _, _, _) in LAYERS:
        Kp = min(P, Cin)
        decl_w(name, 9, _ceil_div(Cin, Kp), Kp, Cout)
    decl_w("v1", 9, 1, TOPK, 16)
    decl_w("v2", 9, 1, 16, 16)
    decl_w("v3", 1, 1, 16, 1)

    zeros = nc.dram_tensor("zeros", (262144,), F32, kind="ExternalInput")
    mask = nc.dram_tensor("mask", (NQ, NP_), F32, kind="ExternalInput")
    wr1 = nc.dram_tensor("wr1", (P, 25), F32, kind="ExternalInput")
    wr1m = nc.dram_tensor("wr1m", (P, 25), F32, kind="ExternalInput")
    wc1 = nc.dram_tensor("wc1", (P, 40), F32, kind="ExternalInput")
    wc1m = nc.dram_tensor("wc1m", (P, 40), F32, kind="ExternalInput")
    r160t = nc.dram_tensor("r160t", (21, 160), F32, kind="ExternalInput")
    c320t = nc.dram_tensor("c320t", (40, 320), F32, kind="ExternalInput")

    out = nc.dram_tensor("out", (160, 320), F32, kind="ExternalOutput")
    dbg = {}
    if DEBUG_OUTPUTS:
        dbg["qs_dbg"] = nc.dram_tensor("qs_dbg", (P, 2, QROWS, HW), F32,
                                       kind="ExternalOutput")
        dbg["valq_dbg"] = nc.dram_tensor("valq_dbg", (TOPK, NQ), F32,
                                         kind="ExternalOutput")
        dbg["r_dbg"] = nc.dram_tensor("r_dbg", (1, NP_), F32,
                                      kind="ExternalOutput")
        dbg["e_dbg"] = nc.dram_tensor("e_dbg", (P, NP_), F32,
                                      kind="ExternalOutput")
        dbg["v1_dbg"] = nc.dram_tensor("v1_dbg", (16, 23, 42), F32,
                                       kind="ExternalOutput")
        dbg["v2_dbg"] = nc.dram_tensor("v2_dbg", (16, 21, 40), F32,
                                       kind="ExternalOutput")
        dbg["o21_dbg"] = nc.dram_tensor("o21_dbg", (21, 40), F32,
                                        kind="ExternalOutput")
        dbg["m1t_dbg"] = nc.dram_tensor("m1t_dbg", (40, 160), F32,
                                        kind="ExternalOutput")

    acts = {}
    for name, (G, Cp, OH, W) in ACTS.items():
        acts[name] = nc.dram_tensor(name, (Cp, G, OH + 1, W + 2), F32)

    odram = nc.dram_tensor("odram", (21, 40), F32)
    fcontrib = nc.dram_tensor("fcontrib", (P, 2, 800), F32)
    fgather = nc.dram_tensor("fgather", (2, P, 2, 800), F32)
    ccontrib = nc.dram_tensor("ccontrib", (800,), F32)
    cgather = nc.dram_tensor("cgather", (2, 800), F32)

    with tile.TileContext(nc) as tc:
        with tc.tile_pool(name="glob", bufs=1) as glob:
            ident = glob.tile([P, P], F32, name="ident")
            make_identity(nc, ident[:])
            ones_col = glob.tile([P, 1], F32, name="ones_col")
            nc.vector.memset(ones_col[:], 1.0)
            ones_row = glob.tile([1, P], F32, name="ones_row")
            nc.vector.memset(ones_row[:], 1.0)

            # ---- zero row 0 of every DRAM act tensor ---------------------
            # (pad *columns* are written by the producers themselves; row 0
            # comes from host zeros via plain contiguous DRAM->DRAM DMAs)
            for name, (G, Cp, OH, W) in ACTS.items():
                if name == "spp":
                    continue            # spp writer covers the whole tensor
                t = acts[name]
                zrow = zeros[0:Cp * G * (W + 2)].rearrange(
                    "(c g h w) -> c g h w", c=Cp, g=G, h=1)
                nc.sync.dma_start(t[:, :, 0:1, :], zrow)

            # ---- conv0 (3->64) via host-padded input, K=27 im2col --------
            _conv0(nc, tc, x0col, w_dr["c0"], b_dr["c0"], acts["a0"])

            # ---- VGG + pyramid convs -------------------------------------
            for (name, Cin, Cout, Rout, W, pool, relu, src, dst) in LAYERS:
                if src == "feat":
                    _build_feat(nc, tc, acts, wr1, wr1m, wc1, wc1m)
                if src == "spp":
                    _build_spp(nc, tc, acts)
                _conv(nc, tc, name, acts[src], acts[dst], w_dr[name],
                      b_dr[name], Cin=Cin, Cout=Cout, Rout=Rout, W=W,
                      pool=pool, relu=relu)

            # ---- correlation / dual softmax / top-k ----------------------
            valq_sb = _corr_topk(nc, tc, glob, acts["qs"], mask,
                                 fcontrib, fgather, ccontrib, cgather,
                                 ones_col, ones_row, ident, groups, dbg)

            # ---- decoder tail + bilinear upsample ------------------------
            _tail(nc, tc, glob, valq_sb, w_dr, b_dr, r160t, c320t, ident,
                  out, odram, dbg)

    nc.finalize()
    return nc


def _conv0(nc, tc, x0col, wdr, bdr, dst):
    """3->64 conv as K=27 im2col matmul (host-built). Out rows [0,261)."""
    Rout, W = 261, 320
    with tc.tile_pool(name="c0w", bufs=1) as wpool, \
         tc.tile_pool(name="c0io", bufs=3) as iopool, \
         tc.tile_pool(name="c0ps", bufs=1, space="PSUM") as pspool:
        wt = wpool.tile([27, 64], F32, name="c0_wt")
        nc.sync.dma_start(wt[:], wdr[0, 0, :, :])
        bt = wpool.tile([64, 1], F32, name="c0_bt")
        nc.sync.dma_start(bt[:], bdr[:, :])
        nr_chunk = 6
        for r0 in range(0, Rout, nr_chunk):
            nr = min(nr_chunk, Rout - r0)
            it = iopool.tile([27, nr_chunk, W], F32, tag="c0_in")
            nc.sync.dma_start(it[:, 0:nr, :], x0col[:, r0:r0 + nr, :])
            ot = iopool.tile([64, nr_chunk, W + 2], F32, tag="c0_out")
            nc.vector.memset(ot[:, 0:nr, 0:1], 0.0)
            nc.vector.memset(ot[:, 0:nr, W + 1:W + 2], 0.0)
            for rp in range(nr):
                ps = pspool.tile([64, W], F32, tag=f"c0ps{rp % 6}", bufs=1)
                nc.tensor.matmul(ps[:], wt[:], it[:, rp, :],
                                 start=True, stop=True)
                nc.scalar.activation(ot[:, rp, 1:1 + W], ps[:], AF.Relu,
                                     bias=bt[:])
            nc.sync.dma_start(dst[:, 0, 1 + r0:1 + r0 + nr, :],
                              ot[:, 0:nr, :])


def _conv(nc, tc, name, src, dst, wdr, bdr, *, Cin, Cout, Rout, W,
          pool, relu):
    """Standard 3x3 pad-1 conv (+opt fused 2x2 maxpool) DRAM->DRAM."""
    Kp = min(P, Cin)
    Kg = _ceil_div(Cin, Kp)
    Mg = _ceil_div(Cout, P)
    nrp = 512 // W                      # rows per psum tile
    nr_chunk = min(Rout, nrp * 6)
    if pool and nr_chunk % 2:
        nr_chunk -= 1
    Wpad = W + 2

    with tc.tile_pool(name=f"{name}w", bufs=1) as wpool, \
         tc.tile_pool(name=f"{name}io", bufs=3) as iopool, \
         tc.tile_pool(name=f"{name}ps", bufs=1, space="PSUM") as pspool:
        # all weight tiles resident for the layer
        wtiles = {}
        for t in range(9):
            for kg in range(Kg):
                for mg in range(Mg):
                    Mp = min(P, Cout - mg * P)
                    w = wpool.tile([Kp, Mp], F32, name=f"{name}w{t}_{kg}_{mg}",
                                   tag=f"{name}w{t}_{kg}_{mg}")
                    nc.sync.dma_start(w[:], wdr[t, kg, :, mg * P:mg * P + Mp])
                    wtiles[(t, kg, mg)] = w
        btiles = []
        for mg in range(Mg):
            Mp = min(P, Cout - mg * P)
            b = wpool.tile([Mp, 1], F32, name=f"{name}b{mg}",
                           tag=f"{name}b{mg}")
            nc.sync.dma_start(b[:], bdr[mg * P:mg * P + Mp, :])
            btiles.append(b)

        for r0 in range(0, Rout, nr_chunk):
            nr = min(nr_chunk, Rout - r0)
            ngroups = _ceil_div(nr, nrp)
            in_bufs = 1 if Rout <= nr_chunk else 2
            it = iopool.tile([Kp, Kg, nr_chunk + 2, Wpad], F32,
                             tag=f"{name}in", bufs=in_bufs)
            nc.sync.dma_start(it[:, :, 0:nr + 2, :],
                              src[:Kp, :, r0:r0 + nr + 2, :])
            for mg in range(Mg):
                Mp = min(P, Cout - mg * P)
                ot = iopool.tile([Mp, nr_chunk, Wpad], F32, tag=f"{name}out")
                if not pool:
                    nc.vector.memset(ot[:, 0:nr, 0:1], 0.0)
                    nc.vector.memset(ot[:, 0:nr, W + 1:W + 2], 0.0)
                pstiles = []
                for gi in range(ngroups):
                    nrow = min(nrp, nr - gi * nrp)
                    ps = pspool.tile([Mp, nrp * W], F32,
                                     tag=f"{name}ps{gi % 6}", bufs=1)
                    pstiles.append((ps, gi * nrp, nrow))
                nmm = 9 * Kg
                i = 0
                for t in range(9):
                    ky, kx = t // 3, t % 3
                    for kg in range(Kg):
                        w = wtiles[(t, kg, mg)]
                        for (ps, rp0, nrow) in pstiles:
                            rhs = it[:, kg, ky + rp0:ky + rp0 + nrow,
                                     kx:kx + W]
                            nc.tensor.matmul(
                                ps[:, 0:nrow * W], w[:], rhs,
                                start=(i == 0), stop=(i == nmm - 1))
                        i += 1
                for (ps, rp0, nrow) in pstiles:
                    nc.scalar.activation(
                        ot[:, rp0:rp0 + nrow, 1:1 + W], ps[:, 0:nrow * W],
                        AF.Relu if relu else AF.Identity, bias=btiles[mg][:])
                if pool:
                    pw = iopool.tile([Mp, nr_chunk, W // 2], F32,
                                     tag=f"{name}pw")
                    nc.vector.tensor_tensor(pw[:, 0:nr, :],
                                            ot[:, 0:nr, 1:1 + W:2],
                                            ot[:, 0:nr, 2:2 + W:2], ALU.max)
                    ph = iopool.tile([Mp, nr_chunk // 2, W // 2 + 2], F32,
                                     tag=f"{name}ph")
                    nc.vector.memset(ph[:, 0:nr // 2, 0:1], 0.0)
                    nc.vector.memset(ph[:, 0:nr // 2,
                                        W // 2 + 1:W // 2 + 2], 0.0)
                    nc.vector.tensor_tensor(ph[:, 0:nr // 2, 1:1 + W // 2],
                                            pw[:, 0:nr:2, :],
                                            pw[:, 1:nr:2, :], ALU.max)
                    nc.sync.dma_start(
                        dst[:Mp, mg, 1 + r0 // 2:1 + (r0 + nr) // 2, :],
                        ph[:, 0:nr // 2, :])
                else:
                    nc.sync.dma_start(
                        dst[:Mp, mg, 1 + r0:1 + r0 + nr, :],
                        ot[:, 0:nr, :])


def _lerp_rows_cols_affine(nc, pool, src_t, nrows_out, ncols_out,
                           wr_bc, wrm_bc, wc_bc, wcm_bc, tag):
    """x1-style resize: rows r0=2i (strided), cols c0=2j (strided)."""
    Cp, Hs, Ws = src_t.shape
    t_r = pool.tile([Cp, nrows_out, Ws], F32, tag=f"{tag}_r")
    tmp = pool.tile([Cp, nrows_out, Ws], F32, tag=f"{tag}_t")
    ev = src_t[:, 0:2 * nrows_out:2, :]
    od = src_t[:, 1:2 * nrows_out:2, :]
    nc.vector.tensor_tensor(tmp[:], od, wr_bc, ALU.mult)
    nc.vector.tensor_tensor(t_r[:], ev, wrm_bc, ALU.mult)
    nc.vector.tensor_add(t_r[:], t_r[:], tmp[:])
    # output tile is W+2 wide with zeroed border columns (DRAM pad layout)
    t_c = pool.tile([Cp, nrows_out, ncols_out + 2], F32, tag=f"{tag}_c")
    tmp2 = pool.tile([Cp, nrows_out, ncols_out], F32, tag=f"{tag}_t2")
    nc.vector.memset(t_c[:, :, 0:1], 0.0)
    nc.vector.memset(t_c[:, :, ncols_out + 1:ncols_out + 2], 0.0)
    evc = t_r[:, :, 0:2 * ncols_out:2]
    odc = t_r[:, :, 1:2 * ncols_out:2]
    nc.vector.tensor_tensor(tmp2[:], odc, wc_bc, ALU.mult)
    nc.vector.tensor_tensor(t_c[:, :, 1:1 + ncols_out], evc, wcm_bc, ALU.mult)
    nc.vector.tensor_add(t_c[:, :, 1:1 + ncols_out],
                         t_c[:, :, 1:1 + ncols_out], tmp2[:])
    return t_c


def _bilinear_idx(n_in, n_out, n_take):
    """align_corners grid (float32, mirrors jnp.linspace)."""
    r = np.linspace(np.float32(0.0), np.float32(n_in - 1), n_out,
                    dtype=np.float32)[:n_take]
    r0 = np.floor(r).astype(np.int32)
    r1 = np.minimum(r0 + 1, n_in - 1)
    w = (r - r0.astype(np.float32)).astype(np.float32)
    return r0, r1, w


def _build_feat(nc, tc, acts, wr1, wr1m, wc1, wc1m):
    """feat = concat[resize(x1 80->40), x2, resize(x3 20->40)] rows [0,25)."""
    feat = acts["feat"]
    a3, a7, a11 = acts["a3"], acts["a7"], acts["a11"]
    with tc.tile_pool(name="featsb", bufs=1) as fp:
        # ---- x2: direct DRAM->DRAM copy of groups 1,2 (rows [0,26)) ------
        nc.sync.dma_start(feat[:, 1:3, 0:26, :], a7[:, :, 0:26, :])

        # ---- x1: (128, 64, 80) -> (128, 25, 40), affine grids ------------
        t1 = fp.tile([P, 50, 80], F32, name="x1_in")
        nc.sync.dma_start(t1[:], a3[:, 0, 1:51, 1:81])
        wrt = fp.tile([P, 25], F32, name="x1_wr")
        nc.sync.dma_start(wrt[:], wr1[:, :])
        wrmt = fp.tile([P, 25], F32, name="x1_wrm")
        nc.sync.dma_start(wrmt[:], wr1m[:, :])
        wct = fp.tile([P, 40], F32, name="x1_wc")
        nc.sync.dma_start(wct[:], wc1[:, :])
        wcmt = fp.tile([P, 40], F32, name="x1_wcm")
        nc.sync.dma_start(wcmt[:], wc1m[:, :])
        x1o = _lerp_rows_cols_affine(
            nc, fp, t1, 25, 40,
            wrt[:, :, None].to_broadcast([P, 25, 80]),
            wrmt[:, :, None].to_broadcast([P, 25, 80]),
            wct[:, None, :].to_broadcast([P, 25, 40]),
            wcmt[:, None, :].to_broadcast([P, 25, 40]), "x1")
        nc.sync.dma_start(feat[:, 0, 1:26, :], x1o[:])

        # ---- x3: (4,128,13,20) -> rows/cols gathered lerp to (25,40) -----
        t3 = fp.tile([P, 4, 13, 20], F32, name="x3_in")
        nc.sync.dma_start(t3[:], a11[:, :, 1:14, 1:21])
        r0s, r1s, rws = _bilinear_idx(20, 40, 25)
        t3r = fp.tile([P, 4, 25, 20], F32, name="x3_r")
        tmp = fp.tile([P, 4, 1, 20], F32, name="x3_tmp")
        for i in range(25):
            w = float(rws[i])
            a, b = int(r0s[i]), int(r1s[i])
            if w == 0.0:
                nc.vector.tensor_copy(t3r[:, :, i, :], t3[:, :, a, :])
            else:
                nc.vector.tensor_scalar_mul(tmp[:, :, 0, :], t3[:, :, b, :], w)
                nc.vector.scalar_tensor_tensor(
                    t3r[:, :, i, :], t3[:, :, a, :], 1.0 - w,
                    tmp[:, :, 0, :], ALU.mult, ALU.add)
        c0s, c1s, cws = _bilinear_idx(20, 40, 40)
        t3c = fp.tile([P, 4, 25, 42], F32, name="x3_c")
        nc.vector.memset(t3c[:, :, :, 0:1], 0.0)
        nc.vector.memset(t3c[:, :, :, 41:42], 0.0)
        tmpc = fp.tile([P, 4, 25], F32, name="x3_tmpc")
        for j in range(40):
            w = float(cws[j])
            a, b = int(c0s[j]), int(c1s[j])
            if w == 0.0:
                nc.vector.tensor_copy(t3c[:, :, :, 1 + j], t3r[:, :, :, a])
            else:
                nc.vector.tensor_scalar_mul(tmpc[:], t3r[:, :, :, b], w)
                nc.vector.scalar_tensor_tensor(
                    t3c[:, :, :, 1 + j], t3r[:, :, :, a], 1.0 - w,
                    tmpc[:], ALU.mult, ALU.add)
        nc.sync.dma_start(feat[:, 3:7, 1:26, :], t3c[:])


def _build_spp(nc, tc, acts):
    """spp = concat[rep4(maxpool4 q), rep2(maxpool2 q), q], rows [0,24)."""
    q, spp = acts["q"], acts["spp"]
    with tc.tile_pool(name="sppsb", bufs=1) as sp:
        qt = sp.tile([P, 2, 25, 42], F32, name="spp_q")
        nc.sync.dma_start(qt[:], q.ap())
        # w-pairs over data cols [1,41)
        aw = sp.tile([P, 2, 25, 20], F32, name="spp_aw")
        nc.vector.tensor_tensor(aw[:], qt[:, :, :, 1:41:2], qt[:, :, :, 2:42:2],
                                ALU.max)
        # p2 = h-pairs of aw over data rows [1,25)
        p2 = sp.tile([P, 2, 12, 20], F32, name="spp_p2")
        nc.vector.tensor_tensor(p2[:], aw[:, :, 1:24:2, :], aw[:, :, 2:25:2, :],
                                ALU.max)
        # p4 = 2x2 pool of p2
        p4w = sp.tile([P, 2, 12, 10], F32, name="spp_p4w")
        nc.vector.tensor_tensor(p4w[:], p2[:, :, :, 0:20:2], p2[:, :, :, 1:20:2],
                                ALU.max)
        p4 = sp.tile([P, 2, 6, 10], F32, name="spp_p4")
        nc.vector.tensor_tensor(p4[:], p4w[:, :, 0:12:2, :], p4w[:, :, 1:12:2, :],
                                ALU.max)
        sppt = sp.tile([P, 6, 25, 42], F32, name="spp_t")
        nc.vector.memset(sppt[:], 0.0)
        p4w_rep = sp.tile([P, 2, 6, 40], F32, name="spp_p4wr")
        p2w_rep = sp.tile([P, 2, 12, 40], F32, name="spp_p2wr")
        for g in range(2):
            # repeat cols, then rows (each op stays <=4 AP dims)
            nc.vector.tensor_copy(
                p4w_rep[:, g].rearrange("c h (d e) -> c h d e", e=4),
                p4[:, g, :, :, None].to_broadcast([P, 6, 10, 4]))
            nc.vector.tensor_copy(
                sppt[:, g, 1:25, 1:41].rearrange("c (a b) w -> c a b w", b=4),
                p4w_rep[:, g, :, None, :].to_broadcast([P, 6, 4, 40]))
            nc.vector.tensor_copy(
                p2w_rep[:, g].rearrange("c h (d e) -> c h d e", e=2),
                p2[:, g, :, :, None].to_broadcast([P, 12, 20, 2]))
            nc.vector.tensor_copy(
                sppt[:, 2 + g, 1:25, 1:41].rearrange("c (a b) w -> c a b w", b=2),
                p2w_rep[:, g, :, None, :].to_broadcast([P, 12, 2, 40]))
            nc.vector.tensor_copy(sppt[:, 4 + g, :, :], qt[:, g, :, :])
        nc.sync.dma_start(spp.ap(), sppt[:])


def _corr_topk(nc, tc, glob, qs, mask, fcontrib, fgather, ccontrib, cgather,
               ones_col, ones_row, ident, groups, dbg=None):
    """Correlation, dual softmax (via exp/rowsum/colsum), top-24 per q."""
    ntiles = _ceil_div(NQ, P)           # 8 tiles, last has 24 rows
    with tc.tile_pool(name="corr", bufs=1) as cp, \
         tc.tile_pool(name="corrio", bufs=2) as cio, \
         tc.tile_pool(name="corrps", bufs=1, space="PSUM") as cps:
        # ---- load qs, normalize ------------------------------------------
        qs_sb = cp.tile([P, 2, QROWS, HW], F32, name="qs_sb")
        nc.sync.dma_start(qs_sb[:], qs[:, :, 1:24, 1:41])
        qs_fl = qs_sb.rearrange("c g h w -> c g (h w)")
        sq = cp.tile([P, 2, NQ], F32, name="sq_sb")
        nc.vector.tensor_tensor(sq[:], qs_fl, qs_fl, ALU.mult)
        n2 = cp.tile([1, NQ], F32, name="n2_sb")
        for half in range(2):
            sl = slice(half * 460, half * 460 + 460)
            ps = cps.tile([1, 460], F32, tag="n2ps")
            nc.tensor.matmul(ps[:], ones_col[:], sq[:, 0, sl], start=True,
                             stop=False)
            nc.tensor.matmul(ps[:], ones_col[:], sq[:, 1, sl], start=False,
                             stop=True)
            nc.scalar.activation(n2[:, sl], ps[:], AF.Identity)
        nc.vector.tensor_scalar_max(n2[:], n2[:], 1e-24)
        nsq = cp.tile([1, NQ], F32, name="nsq_sb")
        nc.scalar.sqrt(nsq[:], n2[:])
        rn = cp.tile([1, NQ], F32, name="rn_sb")
        nc.vector.reciprocal(rn[:], nsq[:])
        # replicate 1/n across partitions via ones_row matmul
        rnrep = cp.tile([P, NQ], F32, name="rnrep_sb")
        for i in range(0, NQ, 460):
            m = min(460, NQ - i)
            ps = cps.tile([P, 460], F32, tag="reps")
            nc.tensor.matmul(ps[:, 0:m], ones_row[:], rn[:, i:i + m],
                             start=True, stop=True)
            nc.scalar.activation(rnrep[:, i:i + m], ps[:, 0:m], AF.Identity)
        f_sb = cp.tile([P, 2, NQ], F32, name="f_sb")
        nc.vector.tensor_tensor(
            f_sb[:], qs_fl, rnrep[:, None, :].to_broadcast([P, 2, NQ]),
            ALU.mult)

        # ---- AllGather normalized halves ---------------------------------
        nc.sync.dma_start(fcontrib.ap(), f_sb[:, :, 0:800])
        nc.gpsimd.collective_compute(
            "AllGather", ALU.bypass, replica_groups=groups,
            ins=[fcontrib.ap()], outs=[fgather.ap()])
        fgs = cp.tile([P, 4, 800], F32, name="fgs_sb")
        for rk in range(2):
            nc.sync.dma_start(fgs[:, rk * 2:rk * 2 + 2, :], fgather[rk])

        # ---- S^T tiles + exp + row sums ----------------------------------
        e_big = cp.tile([P, ntiles * NP_], F32, name="e_big")
        c_sb = cp.tile([P, ntiles], F32, name="c_sb")
        nc.vector.memset(c_sb[:], 1.0)   # rows past the last partial q-tile
        for qt in range(ntiles):
            m = min(P, NQ - qt * P)
            msk = cio.tile([P, NP_], F32, tag="msk")
            nc.sync.dma_start(msk[:m, :], mask[qt * P:qt * P + m, :])
            tmp = cio.tile([P, NP_], F32, tag="stmp")
            for pt in range(4):
                rk, lo = pt // 2, (pt % 2) * 400
                ps = cps.tile([P, 400], F32, tag="sps", bufs=2)
                for g in range(2):
                    nc.tensor.matmul(ps[0:m, :],
                                     f_sb[:, g, qt * P:qt * P + m],
                                     fgs[:, rk * 2 + g, lo:lo + 400],
                                     start=(g == 0), stop=(g == 1))
                nc.vector.tensor_tensor(tmp[0:m, pt * 400:(pt + 1) * 400],
                                        ps[0:m, :],
                                        msk[0:m, pt * 400:(pt + 1) * 400],
                                        ALU.mult)
            nc.scalar.activation(e_big[0:m, qt * NP_:(qt + 1) * NP_],
                                 tmp[0:m, :], AF.Exp,
                                 accum_out=c_sb[0:m, qt:qt + 1])

        # ---- gather column sums (R = gathered C by symmetry) -------------
        for qt in range(7):
            mcon = min(P, 800 - qt * P)
            nc.sync.dma_start(ccontrib[qt * P:qt * P + mcon],
                              c_sb[0:mcon, qt])
        nc.gpsimd.collective_compute(
            "AllGather", ALU.bypass, replica_groups=groups,
            ins=[ccontrib.ap()], outs=[cgather.ap()])
        rrow = cp.tile([1, NP_], F32, name="rrow_sb")
        for rk in range(2):
            nc.sync.dma_start(rrow[:, rk * 800:(rk + 1) * 800],
                              cgather[rk, None, 0:800])
        rrep = cp.tile([P, NP_], F32, name="rrep_sb")
        for i in range(0, NP_, 400):
            ps = cps.tile([P, 400], F32, tag="reps2")
            nc.tensor.matmul(ps[:], ones_row[:], rrow[:, i:i + 400],
                             start=True, stop=True)
            nc.scalar.activation(rrep[:, i:i + 400], ps[:], AF.Identity)
        rinv = cp.tile([P, NP_], F32, name="rinv_sb")
        nc.vector.reciprocal(rinv[:], rrep[:])
        rc = cp.tile([P, ntiles], F32, name="rc_sb")
        nc.vector.reciprocal(rc[:], c_sb[:])
        src_c = cp.tile([P, ntiles], F32, name="src_sb")
        nc.scalar.sqrt(src_c[:], rc[:])

        # ---- xc = (E*sqrt(1/C))^2 * (1/R); top-24 per row; transpose -----
        valq_sb = glob.tile([TOPK, 25, 42], F32, name="valq_sb")
        nc.vector.memset(valq_sb[:], 0.0)
        valq_fl = glob.tile([TOPK, NQ], F32, name="valq_fl")
        for qt in range(ntiles):
            m = min(P, NQ - qt * P)
            xc = cio.tile([P, NP_], F32, tag="xc")
            nc.scalar.activation(xc[0:m, :],
                                 e_big[0:m, qt * NP_:(qt + 1) * NP_],
                                 AF.Square, scale=src_c[0:m, qt:qt + 1])
            nc.vector.tensor_tensor(xc[0:m, :], xc[0:m, :], rinv[0:m, :],
                                    ALU.mult)
            vals = cio.tile([P, 24], F32, tag="vals")
            scr = cio.tile([P, NP_], F32, tag="scr")
            nc.vector.max(out=vals[0:m, 0:8], in_=xc[0:m, :])
            nc.vector.match_replace(out=scr[0:m, :],
                                    in_to_replace=vals[0:m, 0:8],
                                    in_values=xc[0:m, :], imm_value=0.0)
            nc.vector.max(out=vals[0:m, 8:16], in_=scr[0:m, :])
            nc.vector.match_replace(out=scr[0:m, :],
                                    in_to_replace=vals[0:m, 8:16],
                                    in_values=scr[0:m, :], imm_value=0.0)
            nc.vector.max(out=vals[0:m, 16:24], in_=scr[0:m, :])
            pst = cps.tile([24, P], F32, tag="tps")
            nc.tensor.transpose(pst[:, 0:m], vals[0:m, :], ident[0:m, 0:m])
            nc.vector.tensor_copy(valq_fl[:, qt * P:qt * P + m],
                                  pst[0:TOPK, 0:m])
        nc.vector.tensor_copy(valq_sb[:, 1:24, 1:41],
                              valq_fl.rearrange("k (h w) -> k h w", w=HW))
        if dbg:
            nc.sync.dma_start(dbg["qs_dbg"].ap(), qs_sb[:])
            nc.sync.dma_start(dbg["valq_dbg"].ap(), valq_fl[:])
            nc.sync.dma_start(dbg["r_dbg"].ap(), rrow[:])
            nc.sync.dma_start(dbg["e_dbg"].ap(), e_big[:, 0:NP_])
        return valq_sb


def _tail(nc, tc, glob, valq_sb, w_dr, b_dr, r160t, c320t, ident, out,
          odram, dbg=None):
    """v1(3x3,BN,relu) -> v2(3x3) -> v3(1x1) -> bilinear 40->320 rows[0,160)."""
    with tc.tile_pool(name="tailw", bufs=1) as tw, \
         tc.tile_pool(name="tailps", bufs=1, space="PSUM") as tps:
        wv1 = {}
        for t in range(9):
            w = tw.tile([TOPK, 16], F32, name=f"wv1_{t}", tag=f"wv1_{t}")
            nc.sync.dma_start(w[:], w_dr["v1"][t, 0, :, :])
            wv1[t] = w
        bv1 = tw.tile([16, 1], F32, name="bv1")
        nc.sync.dma_start(bv1[:], b_dr["v1"][:, :])
        wv2 = {}
        for t in range(9):
            w = tw.tile([16, 16], F32, name=f"wv2_{t}", tag=f"wv2_{t}")
            nc.sync.dma_start(w[:], w_dr["v2"][t, 0, :, :])
            wv2[t] = w
        bv2 = tw.tile([16, 1], F32, name="bv2")
        nc.sync.dma_start(bv2[:], b_dr["v2"][:, :])
        wv3 = tw.tile([16, 1], F32, name="wv3")
        nc.sync.dma_start(wv3[:], w_dr["v3"][0, 0, :, :])
        bv3 = tw.tile([1, 1], F32, name="bv3")
        nc.sync.dma_start(bv3[:], b_dr["v3"][:, :])

        # v1: out rows [0,22)
        v1_sb = tw.tile([16, 23, 42], F32, name="v1_sb")
        nc.vector.memset(v1_sb[:], 0.0)
        for (rp0, nrow) in ((0, 12), (12, 10)):
            ps = tps.tile([16, 12 * 40], F32, tag="v1ps")
            for t in range(9):
                ky, kx = t // 3, t % 3
                rhs = valq_sb[:, ky + rp0:ky + rp0 + nrow, kx:kx + 40]
                nc.tensor.matmul(ps[:, 0:nrow * 40], wv1[t][:], rhs,
                                 start=(t == 0), stop=(t == 8))
            nc.scalar.activation(v1_sb[:, 1 + rp0:1 + rp0 + nrow, 1:41],
                                 ps[:, 0:nrow * 40], AF.Relu, bias=bv1[:])
        # v2: out rows [0,21), no activation
        v2_sb = tw.tile([16, 21, 40], F32, name="v2_sb")
        for (rp0, nrow) in ((0, 12), (12, 9)):
            ps = tps.tile([16, 12 * 40], F32, tag="v2ps")
            for t in range(9):
                ky, kx = t // 3, t % 3
                rhs = v1_sb[:, ky + rp0:ky + rp0 + nrow, kx:kx + 40]
                nc.tensor.matmul(ps[:, 0:nrow * 40], wv2[t][:], rhs,
                                 start=(t == 0), stop=(t == 8))
            nc.scalar.activation(v2_sb[:, rp0:rp0 + nrow, :],
                                 ps[:, 0:nrow * 40], AF.Identity, bias=bv2[:])
        # v3 (1x1): o (1, 21*40)
        o_sb = tw.tile([1, 840], F32, name="o_sb")
        v2fl = v2_sb.rearrange("c h w -> c (h w)")
        for (c0, n) in ((0, 440), (440, 400)):
            ps = tps.tile([1, 440], F32, tag="v3ps")
            nc.tensor.matmul(ps[:, 0:n], wv3[:], v2fl[:, c0:c0 + n],
                             start=True, stop=True)
            nc.scalar.activation(o_sb[:, c0:c0 + n], ps[:, 0:n], AF.Identity,
                                 bias=bv3[:])
        # reshape (1,840) -> (21,40) via DRAM (partition-crossing SBUF->SBUF
        # DMA produces garbage on hardware)
        nc.sync.dma_start(odram.rearrange("h w -> (h w)")[None, :], o_sb[:])
        o21 = tw.tile([21, 40], F32, name="o21_sb")
        nc.sync.dma_start(o21[:], odram.ap())
        # m1 = R160 @ o  (160,40)
        r1sb = tw.tile([21, 160], F32, name="r1sb")
        nc.sync.dma_start(r1sb[:], r160t.ap())
        c3sb = tw.tile([40, 320], F32, name="c3sb")
        nc.sync.dma_start(c3sb[:], c320t.ap())
        m1t = tw.tile([40, 160], F32, name="m1t_sb")
        for (p0, mp) in ((0, 128), (128, 32)):
            ps = tps.tile([128, 40], F32, tag="m1ps")
            nc.tensor.matmul(ps[0:mp, :], r1sb[:, p0:p0 + mp], o21[:],
                             start=True, stop=True)
            m1part = tw.tile([128, 40], F32, name=f"m1p_{p0}", tag=f"m1p_{p0}")
            nc.scalar.activation(m1part[0:mp, :], ps[0:mp, :], AF.Identity)
            pst = tps.tile([40, 128], F32, tag="m1tps")
            nc.tensor.transpose(pst[:, 0:mp], m1part[0:mp, :], ident[0:mp, 0:mp])
            nc.vector.tensor_copy(m1t[:, p0:p0 + mp], pst[:, 0:mp])
        if dbg:
            nc.sync.dma_start(dbg["v1_dbg"].ap(), v1_sb[:])
            nc.sync.dma_start(dbg["v2_dbg"].ap(), v2_sb[:])
            nc.sync.dma_start(dbg["o21_dbg"].ap(), o21[:])
            nc.sync.dma_start(dbg["m1t_dbg"].ap(), m1t[:])
        # m2 = m1 @ C320^T  (160, 320)
        for (p0, mp) in ((0, 128), (128, 32)):
            ps = tps.tile([128, 320], F32, tag="m2ps")
            nc.tensor.matmul(ps[0:mp, :], m1t[:, p0:p0 + mp], c3sb[:],
                             start=True, stop=True)
            ot = tw.tile([128, 320], F32, name=f"out_{p0}", tag=f"out_{p0}")
            nc.scalar.activation(ot[0:mp, :], ps[0:mp, :], AF.Identity)
            nc.sync.dma_start(out[p0:p0 + mp, :], ot[0:mp, :])


# ---------------------------------------------------------------------------
# host-side input prep
# ---------------------------------------------------------------------------

def _pack_w(w, flip):
    """(Cout,Cin,3,3) -> (9,Kg,Kp,Cout) float32, taps ky*3+kx."""
    w = np.asarray(w, np.float32)
    if flip:
        w = w[:, :, ::-1, :]
    Cout, Cin = w.shape[:2]
    Kp = min(P, Cin)
    Kg = Cin // Kp
    return np.ascontiguousarray(
        w.transpose(2, 3, 1, 0).reshape(9, Kg, Kp, Cout))


def _fold_bn(w, b, g, beta):
    scale = np.asarray(g, np.float32) / np.float32(math.sqrt(1.0 + BN_EPS))
    w = np.asarray(w, np.float32) * scale[:, None, None, None]
    b = np.asarray(b, np.float32) * scale + np.asarray(beta, np.float32)
    return w, b


def _build_mask(role):
    i = np.arange(NQ)
    qh = i // HW if role == 0 else (HW - 1) - i // HW
    qw = i % HW
    j = np.arange(NP_)
    rank, loc = j // 800, j % 800
    ph_f, pw = loc // HW, loc % HW
    ph = np.where(rank == 0, ph_f, (HW - 1) - ph_f)
    sig = 2.0 * (HW * 0.05) ** 2
    gr = np.exp(-((qh[:, None] - ph[None, :]).astype(np.float32) ** 2) / sig)
    gc = np.exp(-((qw[:, None] - pw[None, :]).astype(np.float32) ** 2) / sig)
    return ((1.0 - gr * gc) * ALPHA).astype(np.float32)


def _host_consts():
    r0, r1, w = _bilinear_idx(80, 40, 25)
    assert np.all(r0[:25] == 2 * np.arange(25)), "x1 row grid not affine"
    wr1 = np.broadcast_to(w[None, :], (P, 25)).astype(np.float32).copy()
    wr1m = (1.0 - wr1).astype(np.float32)
    c0, c1, cw = _bilinear_idx(80, 40, 40)
    cw = cw.copy()
    for j in range(40):
        if c0[j] == 2 * j + 1:
            # exact hit on the odd column (happens at the endpoint): select it
            assert cw[j] == 0.0
            cw[j] = 1.0
        else:
            assert c0[j] == 2 * j, f"x1 col grid not affine at {j}"
    wc1 = np.broadcast_to(cw[None, :], (P, 40)).astype(np.float32).copy()
    wc1m = (1.0 - wc1).astype(np.float32)

    # final upsample matrices (rows [0,160) of 320)
    rr0, rr1, rw = _bilinear_idx(HW, 320, 160)
    r160 = np.zeros((160, 21), np.float32)
    for i in range(160):
        r160[i, rr0[i]] += 1.0 - rw[i]
        r160[i, rr1[i]] += rw[i]
    cc0, cc1, cw2 = _bilinear_idx(HW, 320, 320)
    c320 = np.zeros((320, HW), np.float32)
    for i in range(320):
        c320[i, cc0[i]] += 1.0 - cw2[i]
        c320[i, cc1[i]] += cw2[i]
    return (wr1, wr1m, wc1, wc1m,
            np.ascontiguousarray(r160.T), np.ascontiguousarray(c320.T))


def _make_in_maps(xq, params):
    xq = np.asarray(xq, np.float32)
    vgg = params["vgg"]
    wic, bic = _fold_bn(params["in_conv"]["w"], params["in_conv"]["b"],
                        params["in_bn"]["g"], params["in_bn"]["beta"])
    wpyr, bpyr = _fold_bn(params["pyr_conv"]["w"], params["pyr_conv"]["b"],
                          params["pyr_bn"]["g"], params["pyr_bn"]["beta"])
    wv1, bv1 = _fold_bn(params["v1"]["w"], params["v1"]["b"],
                        params["v_bn"]["g"], params["v_bn"]["beta"])
    wv2 = np.asarray(params["v2"]["w"], np.float32)
    bv2 = np.asarray(params["v2"]["b"], np.float32)
    wv3 = np.asarray(params["v3"]["w"], np.float32)
    bv3 = np.asarray(params["v3"]["b"], np.float32)

    wr1, wr1m, wc1, wc1m, r160t, c320t = _host_consts()
    masks = [_build_mask(0), _build_mask(1)]

    # weights per role (flip = role 1)
    role_maps = []
    for role in range(2):
        m = {}
        w0 = np.asarray(vgg[0]["w"], np.float32)
        if role:
            w0 = w0[:, :, ::-1, :]
        m["w_c0"] = np.ascontiguousarray(
            w0.transpose(2, 3, 1, 0).reshape(1, 1, 27, 64))
        m["b_c0"] = np.asarray(vgg[0]["b"], np.float32).reshape(-1, 1)
        for li, (name, *_rest) in enumerate(LAYERS[:11], start=1):
            m[f"w_{name}"] = _pack_w(vgg[li]["w"], role == 1)
            m[f"b_{name}"] = np.asarray(vgg[li]["b"], np.float32).reshape(-1, 1)
        m["w_ic"] = _pack_w(wic, role == 1)
        m["b_ic"] = bic.reshape(-1, 1)
        m["w_pyr"] = _pack_w(wpyr, role == 1)
        m["b_pyr"] = bpyr.reshape(-1, 1)
        m["w_v1"] = _pack_w(wv1, role == 1)
        m["b_v1"] = bv1.reshape(-1, 1)
        m["w_v2"] = _pack_w(wv2, role == 1)
        m["b_v2"] = bv2.reshape(-1, 1)
        m["w_v3"] = np.ascontiguousarray(
            wv3.transpose(2, 3, 1, 0).reshape(1, 1, 16, 1))
        m["b_v3"] = bv3.reshape(-1, 1)
        m["zeros"] = np.zeros((262144,), np.float32)
        m["mask"] = masks[role]
        m["wr1"], m["wr1m"], m["wc1"], m["wc1m"] = wr1, wr1m, wc1, wc1m
        m["r160t"], m["c320t"] = r160t, c320t
        role_maps.append(m)

    in_maps = []
    n_samples = xq.shape[0]
    for b in range(n_samples):
        for role in range(2):
            img = xq[b]
            if role:
                img = img[:, ::-1, :]
            xpad = np.zeros((3, 263, 322), np.float32)
            xpad[:, 1:263, 1:321] = img[:, 0:262, :]
            x0col = np.empty((27, 261, 320), np.float32)
            for ky in range(3):
                for kx in range(3):
                    t = ky * 3 + kx
                    x0col[t * 3:(t + 1) * 3] = xpad[:, ky:ky + 261, kx:kx + 320]
            im = dict(role_maps[role])
            im["x0col"] = x0col
            in_maps.append(im)
    return in_maps


_NC_CACHE = {}


def get_nc(n_cores=8):
    if n_cores not in _NC_CACHE:
        _NC_CACHE[n_cores] = build_nc(n_cores)
    return _NC_CACHE[n_cores]


def kernel(xq, xp, params):
    xq = np.asarray(xq, np.float32)
    n_cores = 2 * xq.shape[0]
    nc = get_nc(n_cores)
    in_maps = _make_in_maps(xq, params)
    res = run_bass_kernel_spmd(nc, in_maps, core_ids=list(range(n_cores)))
    outs = res.results
    full = np.zeros((xq.shape[0], 1, 320, 320), np.float32)
    for b in range(xq.shape[0]):
        full[b, 0, 0:160] = outs[2 * b]["out"]
        full[b, 0, 160:320] = outs[2 * b + 1]["out"][::-1]
    return full
